# revision 42
# baseline (speedup 1.0000x reference)
"""CRF negative log-likelihood loss kernel for Trainium2 (8 NeuronCores).

Math: loss[b] = logsumexp over tag paths (forward algorithm) minus the
gold-path score.  The forward recurrence runs in scaled probability space
(E = exp(trans), per-step offset d = 6.5445):
    S_t = (E^T S_{t-1}) * exp(x_t - d)

Products of random positive matrices contract exponentially, so a 16-step
chunk product is numerically rank-1 (validated: lnZ err ~5e-3 abs on ~3400).
The T=512 scan splits into C=32 chunks of 16 steps; with Gamma_c the chunk-c
operator,
    ln Z = sum_i ln(q_{i+1}^T E^T p_i) - sum_{c interior} ln(1^T p_c) + 512 d
with p_c = Gamma_c 1 (fwd chain) and q_c^T = 1^T Gamma_c (bwd chain,
weights E^T).  All 62 chains (31 fwd + 31 bwd) run concurrently, 15 matmul
rounds of 496 columns per direction; fewer/wider rounds amortize LDWEIGHTS
and per-op overhead vs. a 32-round variant.

Emission factors exp(x-d) live in a CANONICAL buffer (each timestep exp'd
exactly once): col = r*1024 + j*512 + cc*16 + b.  At round r the fwd chains
read the contiguous 496-col j-runs of slice r, the bwd chains those of
slice 15-r (+16 offset), so every state-update multiply is a plain 2D
contiguous op and the upload/exp streams from both ends toward the middle.

Each round's state update is one 3D DVE multiply per direction straight
from PSUM (PSUM reads cap the DVE at 1 elem/cycle/lane, which makes the
DVE the pacing engine at ~2.3us/round; every Scalar/GpSimd drain-assist
variant measured slower because the copy+multiply chain lands on a
direction's critical path).  The PE pre-warms with dummy matmuls so the
HAM clock gate is at 8/8 when the scan starts, and the ln activation
table is pulled in mid-scan by a dummy Ln so the tail pays no reload.

Gold-path score: the host GATHERS x[b,t,y_bt] and trans[y_t,y_t+1] (pure
integer indexing, no float arithmetic) into a [128,128] f32 tile; the device
reduces it.  All float math stays on device.
"""
import numpy as np

B, T, K = 128, 512, 256
NCORES = 8
BS = B // NCORES       # 16 batch rows per core
D_OFF = 6.544520       # per-step log-space offset (mean forward-gain)
CC = 32                # chunks
LC = T // CC           # 16 rounds per chain
NF = CC - 1            # chains per direction (31)
DIRW = NF * 16         # cols per (dir, j) region = 496
XCOLS = LC * 1024      # canonical emission cols = 16384
SC_DRAIN_FROM = 8      # rounds >= this drain dir-1 PSUM on the Scalar engine

_nc_cache = None


def _build_bass():
    import concourse.bass as bass
    import concourse.bacc as bacc
    import concourse.tile as tile
    from concourse import mybir

    f32 = mybir.dt.float32
    bf16 = mybir.dt.bfloat16
    i32 = mybir.dt.int32
    AF = mybir.ActivationFunctionType
    Alu = mybir.AluOpType
    X = mybir.AxisListType.X

    nc = bacc.Bacc()

    xte = nc.declare_dram_parameter("xte", [128, XCOLS], bf16, isOutput=False)
    xg = nc.declare_dram_parameter("xg", [128, 128], f32, isOutput=False)
    tr = nc.declare_dram_parameter("trans", [K, K], f32, isOutput=False)
    trt = nc.declare_dram_parameter("trans_t", [K, K], f32, isOutput=False)
    out = nc.declare_dram_parameter("out", [BS], f32, isOutput=True)

    with tile.TileContext(nc) as tc:
        with (
            tc.tile_pool(name="consts", bufs=1) as consts,
            tc.tile_pool(name="state", bufs=2) as state_p,
            tc.tile_pool(name="psum", bufs=1, space="PSUM") as psum_p,
        ):
            # ---- PE warm-up: ~5us of dummy matmuls so the HAM clock gate
            # reaches 8/8 before the real scan starts (cold PE runs at half
            # clock for its first ~3.4us of activity).  Output aliases the
            # b0 PSUM slot; the WAW dep simply orders round 1 after them.
            warmsb = consts.tile([128, 128], bf16, tag="warmsb")
            nc.vector.memset(warmsb[:], 0.5)
            warmps = psum_p.tile([128, 128], f32, tag="b0", name="warmps")
            for _ in range(34):
                nc.tensor.matmul(out=warmps[:], lhsT=warmsb[:], rhs=warmsb[:],
                                 start=True, stop=True)

            negd = consts.tile([128, 1], f32, tag="negd")
            nc.vector.memset(negd[:], -D_OFF)
            # Dep-light dummy Exp: pulls the exp ACT_TABLE_LOAD into the
            # engine-start idle window, ahead of the DMA-gated real exps.
            expjunk = consts.tile([16, 1], f32, tag="expjunk")
            nc.scalar.activation(out=expjunk[:], in_=warmsb[0:16, 0:1],
                                 func=AF.Exp)

            # ---- the two emission chunks that unblock round 0 go first,
            # then the small weight/gold uploads, then the remaining stream
            # (both ends toward the middle; round r consumes slices r, 15-r).
            xtb = consts.tile([128, XCOLS], bf16, tag="xtb")
            exd = consts.tile([128, XCOLS], bf16, tag="exd")
            chunks = [(0, 1024), (15360, 1024), (1024, 1024), (14336, 1024),
                      (2048, 1024), (13312, 1024), (3072, 1024),
                      (12288, 1024), (4096, 2048), (10240, 2048),
                      (6144, 2048), (8192, 2048)]
            for base, w in chunks[:2]:
                nc.sync.dma_start(out=xtb[:, base:base + w],
                                  in_=xte[:, base:base + w])

            # ---- constants: E = exp(trans), EB = exp(trans^T) in bf16.
            # Both k-halves share one wide tile so each needs only one exp.
            tr_sb = consts.tile([128, 2 * K], f32, tag="tr_sb")
            trt_sb = consts.tile([128, 2 * K], f32, tag="trt_sb")
            for c in range(2):
                nc.sync.dma_start(out=tr_sb[:, c * K:(c + 1) * K],
                                  in_=tr[c * 128:(c + 1) * 128, :])
                nc.sync.dma_start(out=trt_sb[:, c * K:(c + 1) * K],
                                  in_=trt[c * 128:(c + 1) * 128, :])
            e2 = consts.tile([128, 2 * K], bf16, tag="e2")
            nc.scalar.activation(out=e2[:], in_=tr_sb[:], func=AF.Exp)
            eb2 = consts.tile([128, 2 * K], bf16, tag="eb2")
            nc.scalar.activation(out=eb2[:], in_=trt_sb[:], func=AF.Exp)
            def wquad(d, kk, j):
                t = e2 if d == 0 else eb2
                return t[:, kk * K + j * 128:kk * K + (j + 1) * 128]
            ones16 = consts.tile([128, 16], bf16, tag="ones16")
            nc.vector.memset(ones16[:], 1.0)

            # ---- gold-path score: reduce the host-gathered values.
            xg_sb = consts.tile([128, 128], f32, tag="xg")
            nc.sync.dma_start(out=xg_sb[:], in_=xg[:, :])
            pidx = consts.tile([128, 1], i32, tag="pidx")
            nc.gpsimd.iota(pidx[:], pattern=[[0, 1]], base=0,
                           channel_multiplier=1)
            iota16 = consts.tile([128, 16], i32, tag="iota16")
            nc.gpsimd.iota(iota16[:], pattern=[[1, 16]], base=0,
                           channel_multiplier=0)
            pr3 = consts.tile([128, 1], i32, tag="pr3")
            nc.vector.tensor_scalar(pr3[:], pidx[:], 3, None,
                                    Alu.logical_shift_right)
            sel8 = consts.tile([128, 16], f32, tag="sel8")
            nc.vector.tensor_tensor(sel8[:], iota16[:],
                                    pr3[:].to_broadcast([128, 16]), Alu.is_equal)
            xgred = consts.tile([128, 1], f32, tag="xgred")
            nc.vector.tensor_reduce(xgred[:], xg_sb[:], X, Alu.add)

            # ---- finisher masks over [16, 496]: maskC[p, c*16+b] = (b == p),
            # maskI additionally excludes chain position c == 0.
            iota496 = consts.tile([16, DIRW], i32, tag="iota496")
            nc.gpsimd.iota(iota496[:], pattern=[[1, DIRW]], base=0,
                           channel_multiplier=0)
            band = consts.tile([16, DIRW], i32, tag="band")
            nc.vector.tensor_scalar(band[:], iota496[:], 15, None,
                                    Alu.bitwise_and)
            maskC = consts.tile([16, DIRW], f32, tag="maskC")
            nc.vector.tensor_tensor(maskC[:], band[:],
                                    pidx[0:16, :].to_broadcast([16, DIRW]),
                                    Alu.is_equal)
            cidx = consts.tile([16, DIRW], i32, tag="cidx")
            nc.vector.tensor_scalar(cidx[:], iota496[:], 4, None,
                                    Alu.logical_shift_right)
            mnz = consts.tile([16, DIRW], f32, tag="mnz")
            nc.vector.tensor_scalar(mnz[:], cidx[:], 0, None, Alu.not_equal)
            maskI = consts.tile([16, DIRW], f32, tag="maskI")
            nc.vector.tensor_tensor(maskI[:], maskC[:], mnz[:], Alu.mult)

            # ---- remaining upload + exd = exp(x - d) per chunk.
            for base, w in chunks[2:]:
                nc.sync.dma_start(out=xtb[:, base:base + w],
                                  in_=xte[:, base:base + w])
            for base, w in chunks:
                nc.scalar.activation(out=exd[:, base:base + w],
                                     in_=xtb[:, base:base + w],
                                     func=AF.Exp, bias=negd[:])
            # Dummy Ln reading exd (written by chunk-1's exp): the data dep
            # stops the scheduler from running it FIRST, which would load
            # the ln table set ahead of the exp set and delay the whole exp
            # supply chain.  This way the exp table loads during the DMA
            # wait and the ln set loads mid-scan, off both critical paths.
            lnjunk = consts.tile([16, 1], f32, tag="lnjunk")
            nc.scalar.activation(out=lnjunk[:], in_=exd[0:16, 8192:8193],
                                 func=AF.Ln)

            # exd slice for (round r, direction d, j-half): fwd chains
            # cc=0..30 read slice r; bwd chains (chunk p+2) read slice 15-r
            # at a +16 offset.  Always a contiguous 496-col run.
            def exd_run(r, d, j):
                s = r if d == 0 else LC - 1 - r
                base = s * 1024 + j * 512 + (0 if d == 0 else 16)
                return exd[:, base:base + DIRW]

            # ---- no round-0 staging: round 1's matmuls read the init
            # emissions straight out of the canonical exd buffer (each
            # kk-half is a contiguous 496-col run there).
            cur = [None, None]

            def rhs_ap(d, kk, r):
                if r == 1:
                    s = 0 if d == 0 else LC - 1
                    off = 0 if d == 0 else 16
                    base = s * 1024 + kk * 512 + off
                    return exd[:, base:base + DIRW]
                return cur[d][:, kk * DIRW:(kk + 1) * DIRW]

            # ---- the scan: 15 rounds.  PSUM j-regions are 512-padded so
            # each matmul output stays inside one 2KB bank.
            for r in range(1, LC):
                psd = [psum_p.tile([128, 1024], f32, tag=f"b{d}",
                                   name=f"b{d}") for d in range(2)]
                # kk-major order: adjacent matmuls hit different PSUM
                # regions, hiding the systolic drain between the start/stop
                # pair of each accumulation group.
                for d in range(2):
                    for kk in range(2):
                        for j in range(2):
                            nc.tensor.matmul(
                                out=psd[d][:, j * 512:j * 512 + DIRW],
                                lhsT=wquad(d, kk, j),
                                rhs=rhs_ap(d, kk, r),
                                start=(kk == 0), stop=(kk == 1))
                newst = [state_p.tile([128, 2 * DIRW], bf16, tag=f"s{d}",
                                      name=f"s{d}") for d in range(2)]
                # State update: one 3D DVE multiply per direction straight
                # from PSUM.  Every Scalar/GpSimd drain-assist variant was
                # measured SLOWER (2.8-3.6us rounds vs 2.3): the copy+mult
                # chain always lands on some direction's critical path.
                # The last round is j-split so the stitch can start per half.
                for d in range(2):
                    s = r if d == 0 else LC - 1 - r
                    off = 0 if d == 0 else 16
                    if r == LC - 1:
                        for j in range(2):
                            nc.vector.tensor_tensor(
                                newst[d][:, j * DIRW:(j + 1) * DIRW],
                                psd[d][:, j * 512:j * 512 + DIRW],
                                exd[:, s * 1024 + j * 512 + off:
                                     s * 1024 + j * 512 + off + DIRW],
                                Alu.mult)
                        continue
                    ex3 = exd[:, s * 1024:(s + 1) * 1024].rearrange(
                        "p (j x) -> p j x", j=2)[:, :, off:off + DIRW]
                    nc.vector.tensor_tensor(
                        newst[d][:].rearrange("p (j x) -> p j x", j=2),
                        psd[d][:].rearrange("p (j x) -> p j x", j=2)
                        [:, :, 0:DIRW],
                        ex3, Alu.mult)
                cur = [newst[0], newst[1]]

            # ---- interior-sum path: s_c = 1^T p_c for chain positions 1..30.
            csi_ps = psum_p.tile([16, 512], f32, tag="csi")
            for j in range(2):
                nc.tensor.matmul(out=csi_ps[:, 0:DIRW], lhsT=ones16[:],
                                 rhs=cur[0][:, j * DIRW:(j + 1) * DIRW],
                                 start=(j == 0), stop=(j == 1))
            lnI = consts.tile([16, DIRW], f32, tag="lnI")
            nc.scalar.activation(out=lnI[:], in_=csi_ps[:, 0:DIRW], func=AF.Ln)

            # ---- extra matmul round: r_i = E^T p_i for all fwd chains.
            pse = psum_p.tile([128, 1024], f32, tag="pse", name="pse")
            for j in range(2):
                for kk in range(2):
                    nc.tensor.matmul(
                        out=pse[:, j * 512:j * 512 + DIRW],
                        lhsT=wquad(0, kk, j),
                        rhs=cur[0][:, kk * DIRW:(kk + 1) * DIRW],
                        start=(kk == 0), stop=(kk == 1))

            # ---- cross path: chain position i-1 holds both r_i (pse) and
            # q_{i+1} (cur[1]), so two j-split multiplies cover all crosses.
            crossm = consts.tile([128, 2 * DIRW], bf16, tag="crossm")
            csc_ps = psum_p.tile([16, 512], f32, tag="csc")
            for j in range(2):
                nc.vector.tensor_tensor(crossm[:, j * DIRW:(j + 1) * DIRW],
                                        pse[:, j * 512:j * 512 + DIRW],
                                        cur[1][:, j * DIRW:(j + 1) * DIRW],
                                        Alu.mult)
                nc.tensor.matmul(out=csc_ps[:, 0:DIRW], lhsT=ones16[:],
                                 rhs=crossm[:, j * DIRW:(j + 1) * DIRW],
                                 start=(j == 0), stop=(j == 1))
            # gold-path fold shares the csc bank (separate accum group).
            nc.tensor.matmul(out=csc_ps[:, 496:497], lhsT=sel8[:],
                             rhs=xgred[:], start=True, stop=True)
            lnC = consts.tile([16, DIRW], f32, tag="lnC")
            nc.scalar.activation(out=lnC[:], in_=csc_ps[:, 0:DIRW], func=AF.Ln)
            # interior fused mask+reduce runs on GpSimd (SBUF-only inputs,
            # ample slack) so it cannot steal the DVE from the cross path.
            lnIm = consts.tile([16, DIRW], f32, tag="lnIm")
            ired = consts.tile([16, 1], f32, tag="ired")
            nc.vector.scalar_tensor_tensor(lnIm[:], lnI[:], 0.0, maskI[:],
                                           Alu.bypass, Alu.mult,
                                           accum_out=ired[:])
            # pre-fold the two subtrahends off the critical path
            isum = consts.tile([16, 1], f32, tag="isum")
            nc.vector.tensor_tensor(isum[:], ired[:], csc_ps[:, 496:497],
                                    Alu.add)
            lnCm = consts.tile([16, DIRW], f32, tag="lnCm")
            cred = consts.tile([16, 1], f32, tag="cred")
            nc.vector.scalar_tensor_tensor(lnCm[:], lnC[:], 0.0, maskC[:],
                                           Alu.bypass, Alu.mult,
                                           accum_out=cred[:])

            # ---- loss = sum ln cross - sum ln s + 512 d - target
            loss = consts.tile([16, 1], f32, tag="loss")
            nc.vector.scalar_tensor_tensor(loss[:], cred[:],
                                           float(T) * D_OFF, isum[:],
                                           Alu.add, Alu.subtract)
            nc.sync.dma_start(out=out[:], in_=loss[:, 0:1])

    nc.finalize()
    return nc


def _get_nc():
    global _nc_cache
    if _nc_cache is None:
        _nc_cache = _build_bass()
    return _nc_cache


def _host_prep(y_pred, trans, y_true):
    """Per-core input tensors. Index work only; no float math on inputs."""
    import ml_dtypes

    bf = ml_dtypes.bfloat16

    trans32 = np.ascontiguousarray(np.asarray(trans, dtype=np.float32))
    trans_t = np.ascontiguousarray(trans32.T)
    y32 = np.asarray(y_true).astype(np.int32)
    yp = np.asarray(y_pred, dtype=np.float32)

    bi = np.arange(BS)[:, None]
    ti = np.arange(T)[None, :]
    in_maps = []
    for c in range(NCORES):
        rows = yp[c * BS:(c + 1) * BS]               # [16, T, 256]
        ys = y32[c * BS:(c + 1) * BS]                # [16, T]
        # canonical: xte[klo, r*1024 + j*512 + cc*16 + b]
        #          = x[b, cc*16+r, j*128+klo]
        a = rows.reshape(BS, CC, LC, 2, 128)         # [b, cc, r, j, klo]
        xte = np.ascontiguousarray(a.transpose(4, 2, 3, 1, 0)).reshape(
            128, XCOLS).astype(bf)

        # gold-path values, gathered by index: 512 emissions + 511
        # transitions + 1 zero pad per batch row -> [b*8+s, 128]
        pv = rows[bi, ti, ys]                        # [16, 512]
        tv = trans32[ys[:, :-1], ys[:, 1:]]          # [16, 511]
        vals = np.concatenate(
            [pv, tv, np.zeros((BS, 1), np.float32)], axis=1)  # [16, 1024]
        xgv = np.ascontiguousarray(vals.reshape(BS * 8, 128))

        in_maps.append({"xte": xte, "xg": xgv,
                        "trans": trans32, "trans_t": trans_t})
    return in_maps


LAST_EXEC_TIME_NS = None


def kernel(y_pred, trans, y_true):
    import os
    from concourse.bass_utils import run_bass_kernel_spmd

    global LAST_EXEC_TIME_NS

    in_maps = _host_prep(y_pred, trans, y_true)
    nc = _get_nc()
    trace = bool(int(os.environ.get("CRF_KERNEL_TRACE", "0")))
    for attempt in range(3):
        res = run_bass_kernel_spmd(
            nc, in_maps, core_ids=list(range(NCORES)), trace=trace
        )
        LAST_EXEC_TIME_NS = res.exec_time_ns
        out_full = np.concatenate(
            [res.results[i]["out"].reshape(BS) for i in range(NCORES)]
        ).astype(np.float32)
        # The math guarantees finite losses; a non-finite value means a rare
        # execution-level fault, so rerun.
        if np.isfinite(out_full).all():
            return out_full
    return out_full


# revision 43
# speedup vs baseline: 1.0047x; 1.0047x over previous
"""CRF negative log-likelihood loss kernel for Trainium2 (8 NeuronCores).

Math: loss[b] = logsumexp over tag paths (forward algorithm) minus the
gold-path score.  The forward recurrence runs in scaled probability space
(E = exp(trans), per-step offset d = 6.5445):
    S_t = (E^T S_{t-1}) * exp(x_t - d)

Products of random positive matrices contract exponentially, so a 16-step
chunk product is numerically rank-1 (validated: lnZ err ~5e-3 abs on ~3400).
The T=512 scan splits into C=32 chunks of 16 steps; with Gamma_c the chunk-c
operator,
    ln Z = sum_i ln(q_{i+1}^T E^T p_i) - sum_{c interior} ln(1^T p_c) + 512 d
with p_c = Gamma_c 1 (fwd chain) and q_c^T = 1^T Gamma_c (bwd chain,
weights E^T).  All 62 chains (31 fwd + 31 bwd) run concurrently, 15 matmul
rounds of 496 columns per direction; fewer/wider rounds amortize LDWEIGHTS
and per-op overhead vs. a 32-round variant.

Emission factors exp(x-d) live in a CANONICAL buffer (each timestep exp'd
exactly once): col = r*1024 + j*512 + cc*16 + b.  At round r the fwd chains
read the contiguous 496-col j-runs of slice r, the bwd chains those of
slice 15-r (+16 offset), so every state-update multiply is a plain 2D
contiguous op and the upload/exp streams from both ends toward the middle.

Each round's state update is one 3D DVE multiply per direction straight
from PSUM (PSUM reads cap the DVE at 1 elem/cycle/lane, which makes the
DVE the pacing engine at ~2.3us/round; every Scalar/GpSimd drain-assist
variant measured slower because the copy+multiply chain lands on a
direction's critical path).  The PE pre-warms with dummy matmuls so the
HAM clock gate is at 8/8 when the scan starts, and the ln activation
table is pulled in mid-scan by a dummy Ln so the tail pays no reload.

Gold-path score: the host GATHERS x[b,t,y_bt] and trans[y_t,y_t+1] (pure
integer indexing, no float arithmetic) into a [128,128] f32 tile; the device
reduces it.  All float math stays on device.
"""
import numpy as np

B, T, K = 128, 512, 256
NCORES = 8
BS = B // NCORES       # 16 batch rows per core
D_OFF = 6.544520       # per-step log-space offset (mean forward-gain)
CC = 32                # chunks
LC = T // CC           # 16 rounds per chain
NF = CC - 1            # chains per direction (31)
DIRW = NF * 16         # cols per (dir, j) region = 496
XCOLS = LC * 1024      # canonical emission cols = 16384
SC_DRAIN_FROM = 8      # rounds >= this drain dir-1 PSUM on the Scalar engine

_nc_cache = None


def _build_bass():
    import concourse.bass as bass
    import concourse.bacc as bacc
    import concourse.tile as tile
    from concourse import mybir

    f32 = mybir.dt.float32
    bf16 = mybir.dt.bfloat16
    i32 = mybir.dt.int32
    AF = mybir.ActivationFunctionType
    Alu = mybir.AluOpType
    X = mybir.AxisListType.X

    nc = bacc.Bacc()

    xte = nc.declare_dram_parameter("xte", [128, XCOLS], bf16, isOutput=False)
    xg = nc.declare_dram_parameter("xg", [128, 128], f32, isOutput=False)
    tr = nc.declare_dram_parameter("trans", [K, K], f32, isOutput=False)
    trt = nc.declare_dram_parameter("trans_t", [K, K], f32, isOutput=False)
    out = nc.declare_dram_parameter("out", [BS], f32, isOutput=True)

    with tile.TileContext(nc) as tc:
        with (
            tc.tile_pool(name="consts", bufs=1) as consts,
            tc.tile_pool(name="state", bufs=2) as state_p,
            tc.tile_pool(name="psum", bufs=1, space="PSUM") as psum_p,
        ):
            # ---- PE warm-up: ~5us of dummy matmuls so the HAM clock gate
            # reaches 8/8 before the real scan starts (cold PE runs at half
            # clock for its first ~3.4us of activity).  Output aliases the
            # b0 PSUM slot; the WAW dep simply orders round 1 after them.
            warmsb = consts.tile([128, 128], bf16, tag="warmsb")
            nc.vector.memset(warmsb[:], 0.5)
            warmps = psum_p.tile([128, 128], f32, tag="b0", name="warmps")
            for _ in range(34):
                nc.tensor.matmul(out=warmps[:], lhsT=warmsb[:], rhs=warmsb[:],
                                 start=True, stop=True)

            negd = consts.tile([128, 1], f32, tag="negd")
            nc.vector.memset(negd[:], -D_OFF)
            # Dep-light dummy Exp: pulls the exp ACT_TABLE_LOAD into the
            # engine-start idle window, ahead of the DMA-gated real exps.
            expjunk = consts.tile([16, 1], f32, tag="expjunk")
            nc.scalar.activation(out=expjunk[:], in_=warmsb[0:16, 0:1],
                                 func=AF.Exp)

            # ---- the two emission chunks that unblock round 0 go first,
            # then the small weight/gold uploads, then the remaining stream
            # (both ends toward the middle; round r consumes slices r, 15-r).
            xtb = consts.tile([128, XCOLS], bf16, tag="xtb")
            exd = consts.tile([128, XCOLS], bf16, tag="exd")
            chunks = [(0, 1024), (15360, 1024), (1024, 1024), (14336, 1024),
                      (2048, 1024), (13312, 1024), (3072, 1024),
                      (12288, 1024), (4096, 2048), (10240, 2048),
                      (6144, 2048), (8192, 2048)]
            for base, w in chunks[:2]:
                nc.sync.dma_start(out=xtb[:, base:base + w],
                                  in_=xte[:, base:base + w])

            # ---- constants: E = exp(trans), EB = exp(trans^T) in bf16.
            # Both k-halves share one wide tile so each needs only one exp.
            tr_sb = consts.tile([128, 2 * K], f32, tag="tr_sb")
            trt_sb = consts.tile([128, 2 * K], f32, tag="trt_sb")
            for c in range(2):
                nc.sync.dma_start(out=tr_sb[:, c * K:(c + 1) * K],
                                  in_=tr[c * 128:(c + 1) * 128, :])
                nc.sync.dma_start(out=trt_sb[:, c * K:(c + 1) * K],
                                  in_=trt[c * 128:(c + 1) * 128, :])
            e2 = consts.tile([128, 2 * K], bf16, tag="e2")
            nc.scalar.activation(out=e2[:], in_=tr_sb[:], func=AF.Exp)
            eb2 = consts.tile([128, 2 * K], bf16, tag="eb2")
            nc.scalar.activation(out=eb2[:], in_=trt_sb[:], func=AF.Exp)
            def wquad(d, kk, j):
                t = e2 if d == 0 else eb2
                return t[:, kk * K + j * 128:kk * K + (j + 1) * 128]
            ones16 = consts.tile([128, 16], bf16, tag="ones16")
            nc.vector.memset(ones16[:], 1.0)

            # ---- gold-path score: reduce the host-gathered values.
            xg_sb = consts.tile([128, 128], f32, tag="xg")
            nc.sync.dma_start(out=xg_sb[:], in_=xg[:, :])
            pidx = consts.tile([128, 1], i32, tag="pidx")
            nc.gpsimd.iota(pidx[:], pattern=[[0, 1]], base=0,
                           channel_multiplier=1)
            iota16 = consts.tile([128, 16], i32, tag="iota16")
            nc.gpsimd.iota(iota16[:], pattern=[[1, 16]], base=0,
                           channel_multiplier=0)
            pr3 = consts.tile([128, 1], i32, tag="pr3")
            nc.vector.tensor_scalar(pr3[:], pidx[:], 3, None,
                                    Alu.logical_shift_right)
            sel8 = consts.tile([128, 16], f32, tag="sel8")
            nc.vector.tensor_tensor(sel8[:], iota16[:],
                                    pr3[:].to_broadcast([128, 16]), Alu.is_equal)
            xgred = consts.tile([128, 1], f32, tag="xgred")
            nc.vector.tensor_reduce(xgred[:], xg_sb[:], X, Alu.add)

            # ---- finisher masks over [16, 496]: maskC[p, c*16+b] = (b == p),
            # maskI additionally excludes chain position c == 0.
            iota496 = consts.tile([16, DIRW], i32, tag="iota496")
            nc.gpsimd.iota(iota496[:], pattern=[[1, DIRW]], base=0,
                           channel_multiplier=0)
            band = consts.tile([16, DIRW], i32, tag="band")
            nc.vector.tensor_scalar(band[:], iota496[:], 15, None,
                                    Alu.bitwise_and)
            maskC = consts.tile([16, DIRW], f32, tag="maskC")
            nc.vector.tensor_tensor(maskC[:], band[:],
                                    pidx[0:16, :].to_broadcast([16, DIRW]),
                                    Alu.is_equal)
            cidx = consts.tile([16, DIRW], i32, tag="cidx")
            nc.vector.tensor_scalar(cidx[:], iota496[:], 4, None,
                                    Alu.logical_shift_right)
            mnz = consts.tile([16, DIRW], f32, tag="mnz")
            nc.vector.tensor_scalar(mnz[:], cidx[:], 0, None, Alu.not_equal)
            maskI = consts.tile([16, DIRW], f32, tag="maskI")
            nc.vector.tensor_tensor(maskI[:], maskC[:], mnz[:], Alu.mult)

            # ---- remaining upload + exd = exp(x - d) per chunk.
            for base, w in chunks[2:]:
                nc.sync.dma_start(out=xtb[:, base:base + w],
                                  in_=xte[:, base:base + w])
            for base, w in chunks:
                nc.scalar.activation(out=exd[:, base:base + w],
                                     in_=xtb[:, base:base + w],
                                     func=AF.Exp, bias=negd[:])
            # Dummy Ln reading exd (written by chunk-1's exp): the data dep
            # stops the scheduler from running it FIRST, which would load
            # the ln table set ahead of the exp set and delay the whole exp
            # supply chain.  This way the exp table loads during the DMA
            # wait and the ln set loads mid-scan, off both critical paths.
            lnjunk = consts.tile([16, 1], f32, tag="lnjunk")
            nc.scalar.activation(out=lnjunk[:], in_=exd[0:16, 8192:8193],
                                 func=AF.Ln)

            # exd slice for (round r, direction d, j-half): fwd chains
            # cc=0..30 read slice r; bwd chains (chunk p+2) read slice 15-r
            # at a +16 offset.  Always a contiguous 496-col run.
            def exd_run(r, d, j):
                s = r if d == 0 else LC - 1 - r
                base = s * 1024 + j * 512 + (0 if d == 0 else 16)
                return exd[:, base:base + DIRW]

            # ---- no round-0 staging: round 1's matmuls read the init
            # emissions straight out of the canonical exd buffer (each
            # kk-half is a contiguous 496-col run there).
            cur = [None, None]

            def rhs_ap(d, kk, r):
                if r == 1:
                    s = 0 if d == 0 else LC - 1
                    off = 0 if d == 0 else 16
                    base = s * 1024 + kk * 512 + off
                    return exd[:, base:base + DIRW]
                return cur[d][:, kk * DIRW:(kk + 1) * DIRW]

            # ---- the scan: 15 rounds.  PSUM j-regions are 512-padded so
            # each matmul output stays inside one 2KB bank.
            for r in range(1, LC):
                psd = [psum_p.tile([128, 1024], f32, tag=f"b{d}",
                                   name=f"b{d}") for d in range(2)]
                # kk-major order: adjacent matmuls hit different PSUM
                # regions, hiding the systolic drain between the start/stop
                # pair of each accumulation group.
                for d in range(2):
                    for kk in range(2):
                        for j in range(2):
                            nc.tensor.matmul(
                                out=psd[d][:, j * 512:j * 512 + DIRW],
                                lhsT=wquad(d, kk, j),
                                rhs=rhs_ap(d, kk, r),
                                start=(kk == 0), stop=(kk == 1))
                newst = [state_p.tile([128, 2 * DIRW], bf16, tag=f"s{d}",
                                      name=f"s{d}") for d in range(2)]
                # State update: one 3D DVE multiply per direction straight
                # from PSUM.  Every Scalar/GpSimd drain-assist variant was
                # measured SLOWER (2.8-3.6us rounds vs 2.3): the copy+mult
                # chain always lands on some direction's critical path.
                # The last round is j-split so the stitch can start per half.
                for d in range(2):
                    s = r if d == 0 else LC - 1 - r
                    off = 0 if d == 0 else 16
                    if r == LC - 1:
                        for j in range(2):
                            nc.vector.tensor_tensor(
                                newst[d][:, j * DIRW:(j + 1) * DIRW],
                                psd[d][:, j * 512:j * 512 + DIRW],
                                exd[:, s * 1024 + j * 512 + off:
                                     s * 1024 + j * 512 + off + DIRW],
                                Alu.mult)
                        continue
                    ex3 = exd[:, s * 1024:(s + 1) * 1024].rearrange(
                        "p (j x) -> p j x", j=2)[:, :, off:off + DIRW]
                    nc.vector.tensor_tensor(
                        newst[d][:].rearrange("p (j x) -> p j x", j=2),
                        psd[d][:].rearrange("p (j x) -> p j x", j=2)
                        [:, :, 0:DIRW],
                        ex3, Alu.mult)
                cur = [newst[0], newst[1]]

            # ---- interior-sum path: s_c = 1^T p_c for chain positions 1..30.
            csi_ps = psum_p.tile([16, 512], f32, tag="csi")
            for j in range(2):
                nc.tensor.matmul(out=csi_ps[:, 0:DIRW], lhsT=ones16[:],
                                 rhs=cur[0][:, j * DIRW:(j + 1) * DIRW],
                                 start=(j == 0), stop=(j == 1))
            lnI = consts.tile([16, DIRW], f32, tag="lnI")
            nc.scalar.activation(out=lnI[:], in_=csi_ps[:, 0:DIRW], func=AF.Ln)

            # ---- extra matmul round: r_i = E^T p_i for all fwd chains.
            pse = psum_p.tile([128, 1024], f32, tag="pse", name="pse")
            for j in range(2):
                for kk in range(2):
                    nc.tensor.matmul(
                        out=pse[:, j * 512:j * 512 + DIRW],
                        lhsT=wquad(0, kk, j),
                        rhs=cur[0][:, kk * DIRW:(kk + 1) * DIRW],
                        start=(kk == 0), stop=(kk == 1))

            # ---- cross path: chain position i-1 holds both r_i (pse) and
            # q_{i+1} (cur[1]), so two j-split multiplies cover all crosses.
            crossm = consts.tile([128, 2 * DIRW], bf16, tag="crossm")
            csc_ps = psum_p.tile([16, 512], f32, tag="csc")
            for j in range(2):
                nc.vector.tensor_tensor(crossm[:, j * DIRW:(j + 1) * DIRW],
                                        pse[:, j * 512:j * 512 + DIRW],
                                        cur[1][:, j * DIRW:(j + 1) * DIRW],
                                        Alu.mult)
                nc.tensor.matmul(out=csc_ps[:, 0:DIRW], lhsT=ones16[:],
                                 rhs=crossm[:, j * DIRW:(j + 1) * DIRW],
                                 start=(j == 0), stop=(j == 1))
            # gold-path fold shares the csc bank (separate accum group).
            nc.tensor.matmul(out=csc_ps[:, 496:497], lhsT=sel8[:],
                             rhs=xgred[:], start=True, stop=True)
            lnC = consts.tile([16, DIRW], f32, tag="lnC")
            nc.scalar.activation(out=lnC[:], in_=csc_ps[:, 0:DIRW], func=AF.Ln)
            lnCm = consts.tile([16, DIRW], f32, tag="lnCm")
            cred = consts.tile([16, 1], f32, tag="cred")
            nc.vector.scalar_tensor_tensor(lnCm[:], lnC[:], 0.0, maskC[:],
                                           Alu.bypass, Alu.mult,
                                           accum_out=cred[:])
            # interior fused mask+reduce emitted LAST so the scheduler
            # slots it into the DVE idle window under LN-C instead of
            # between the cross multiplies (observed hoist).
            lnIm = consts.tile([16, DIRW], f32, tag="lnIm")
            ired = consts.tile([16, 1], f32, tag="ired")
            nc.vector.scalar_tensor_tensor(lnIm[:], lnI[:], 0.0, maskI[:],
                                           Alu.bypass, Alu.mult,
                                           accum_out=ired[:])
            # pre-fold the two subtrahends off the critical path
            isum = consts.tile([16, 1], f32, tag="isum")
            nc.vector.tensor_tensor(isum[:], ired[:], csc_ps[:, 496:497],
                                    Alu.add)

            # ---- loss = sum ln cross - sum ln s + 512 d - target
            loss = consts.tile([16, 1], f32, tag="loss")
            nc.vector.scalar_tensor_tensor(loss[:], cred[:],
                                           float(T) * D_OFF, isum[:],
                                           Alu.add, Alu.subtract)
            nc.sync.dma_start(out=out[:], in_=loss[:, 0:1])

    nc.finalize()
    return nc


def _get_nc():
    global _nc_cache
    if _nc_cache is None:
        _nc_cache = _build_bass()
    return _nc_cache


def _host_prep(y_pred, trans, y_true):
    """Per-core input tensors. Index work only; no float math on inputs."""
    import ml_dtypes

    bf = ml_dtypes.bfloat16

    trans32 = np.ascontiguousarray(np.asarray(trans, dtype=np.float32))
    trans_t = np.ascontiguousarray(trans32.T)
    y32 = np.asarray(y_true).astype(np.int32)
    yp = np.asarray(y_pred, dtype=np.float32)

    bi = np.arange(BS)[:, None]
    ti = np.arange(T)[None, :]
    in_maps = []
    for c in range(NCORES):
        rows = yp[c * BS:(c + 1) * BS]               # [16, T, 256]
        ys = y32[c * BS:(c + 1) * BS]                # [16, T]
        # canonical: xte[klo, r*1024 + j*512 + cc*16 + b]
        #          = x[b, cc*16+r, j*128+klo]
        a = rows.reshape(BS, CC, LC, 2, 128)         # [b, cc, r, j, klo]
        xte = np.ascontiguousarray(a.transpose(4, 2, 3, 1, 0)).reshape(
            128, XCOLS).astype(bf)

        # gold-path values, gathered by index: 512 emissions + 511
        # transitions + 1 zero pad per batch row -> [b*8+s, 128]
        pv = rows[bi, ti, ys]                        # [16, 512]
        tv = trans32[ys[:, :-1], ys[:, 1:]]          # [16, 511]
        vals = np.concatenate(
            [pv, tv, np.zeros((BS, 1), np.float32)], axis=1)  # [16, 1024]
        xgv = np.ascontiguousarray(vals.reshape(BS * 8, 128))

        in_maps.append({"xte": xte, "xg": xgv,
                        "trans": trans32, "trans_t": trans_t})
    return in_maps


LAST_EXEC_TIME_NS = None


def kernel(y_pred, trans, y_true):
    import os
    from concourse.bass_utils import run_bass_kernel_spmd

    global LAST_EXEC_TIME_NS

    in_maps = _host_prep(y_pred, trans, y_true)
    nc = _get_nc()
    trace = bool(int(os.environ.get("CRF_KERNEL_TRACE", "0")))
    for attempt in range(3):
        res = run_bass_kernel_spmd(
            nc, in_maps, core_ids=list(range(NCORES)), trace=trace
        )
        LAST_EXEC_TIME_NS = res.exec_time_ns
        out_full = np.concatenate(
            [res.results[i]["out"].reshape(BS) for i in range(NCORES)]
        ).astype(np.float32)
        # The math guarantees finite losses; a non-finite value means a rare
        # execution-level fault, so rerun.
        if np.isfinite(out_full).all():
            return out_full
    return out_full
